# revision 12
# baseline (speedup 1.0000x reference)
"""Mamba-1 selective-scan recurrence kernel for Trainium2 (8 NeuronCores).

Problem: B=2, L=2048, D=1024, N=16, R=64 (f32).
  x_dbl = hidden @ W_xproj.T ; dt_low, Bm, Cm = split(x_dbl, [R, N, N])
  delta = softplus(dt_low @ W_dt.T + b_dt)
  h_t   = exp(delta_t*A) * h_{t-1} + (delta_t*x_t) * B_t ;  y_t = sum_n C_t(n) h_t(:,n)

Sharding: core = (batch b in {0,1}) x (channel quarter ds in {0..3}); each core
computes y for its 256 channels of one batch.  No cross-core communication.
The per-core input `x` is hidden[b] with columns permuted so the core's own
channel slice sits at columns [0:256].

v2 (vs baseline at 341 us):
  - da (scan decay) in f32: bf16 decay errors compound multiplicatively over
    ~1k steps and blew the 2e-2 budget (baseline measured rel-err 2.3e-2).
  - fp32r projection weights: 1 PE cycle/row instead of 4.
  - phase 1 streams the transposed X per 512-chunk (no 8 MiB persistent XT).
  - B/C broadcast tiles (bf16, per half-L) staged via PE selector matmuls,
    with the PSUM->SBUF copies split between ACT and gpsimd (Pool);
    staging for the second half overlaps the first half's recurrence.
  - engine rebalance: DVE keeps the scans (the hard floor: ~2.26 ns/col,
    no fast modes) + most muls; Pool takes part of ch muls, staging copies,
    and the hend carries; ACT keeps exp/softplus/da.
"""

import sys

for _p in ("/opt/trn_rl_repo",):
    if _p not in sys.path:
        sys.path.insert(0, _p)

import numpy as np

import concourse.bass as bass  # noqa: F401
import concourse.tile as tile
from concourse import bacc, mybir
from concourse.bass_utils import run_bass_kernel_spmd

F32 = mybir.dt.float32
F32R = mybir.dt.float32r
BF16 = mybir.dt.bfloat16

B, L, D, N, R = 2, 2048, 1024, 16, 64
NCORES = 8
DSH = D // 4          # channels per core
P = 128               # partitions
NDT = DSH // P        # d-tiles per core (2)
E = R + 2 * N         # x_dbl feature dim (96)
HLF = L // 2

# --- engine-assignment knobs (tuned against the perfetto trace) ---
CH_POOL_EVERY = 4     # ch-mul goes to Pool when n % CH_POOL_EVERY == 0
UT_ON_DVE = True      # uT mul on DVE (else Pool)
STG1_POOL_EVERY = 0   # phase-1 staging copy to Pool when k>0 and (n%k)==0
STG2_ON_POOL = False  # phase-2 staging copies on Pool (else ACT)

_CACHE = {}


def build_nc(Lc=L):
    nc = bacc.Bacc("TRN2", target_bir_lowering=False, debug=False,
                   num_devices=NCORES)

    x_d = nc.dram_tensor("x", [Lc, D], F32, kind="ExternalInput")
    wxT_d = nc.dram_tensor("wxT", [D, E], F32R, kind="ExternalInput")
    wdtT_d = nc.dram_tensor("wdtT", [R, DSH], F32R, kind="ExternalInput")
    bdt_d = nc.dram_tensor("bdt", [DSH, 1], F32, kind="ExternalInput")
    acol_d = nc.dram_tensor("acol", [DSH, N], F32, kind="ExternalInput")
    ident_d = nc.dram_tensor("ident", [P, P], F32, kind="ExternalInput")
    identa_d = nc.dram_tensor("identa", [P, P], BF16, kind="ExternalInput")
    selbc_d = nc.dram_tensor("selbc", [2 * N, 2 * N * P], F32R,
                             kind="ExternalInput")
    y_d = nc.dram_tensor("y", [Lc, DSH], F32, kind="ExternalOutput")

    with tile.TileContext(nc) as tc:
        _emit(tc, nc, x_d, wxT_d, wdtT_d, bdt_d, acol_d, ident_d, identa_d,
              selbc_d, y_d, Lc)
    nc.compile()
    return nc


def _emit(tc, nc, x_d, wxT_d, wdtT_d, bdt_d, acol_d, ident_d, identa_d,
          selbc_d, y_d, Lc):
    mult = mybir.AluOpType.mult
    add = mybir.AluOpType.add
    AF = mybir.ActivationFunctionType

    NJ = D // P           # full-width d-tiles (8)
    CH = 512
    NCHK = Lc // CH       # phase-1 chunks (4)
    NTH = HLF // P        # time chunks per half (8)

    with (
        tc.tile_pool(name="persist", bufs=1) as persist,
        tc.tile_pool(name="consts", bufs=1) as consts,
        tc.tile_pool(name="bbp", bufs=N + 2) as bbp,
        tc.tile_pool(name="ccp", bufs=N + 2) as ccp,
    ):
        ident = consts.tile([P, P], F32, tag="ident")
        nc.sync.dma_start(ident[:], ident_d[:])
        identa = consts.tile([P, P], BF16, tag="identa")
        nc.sync.dma_start(identa[:], identa_d[:])
        acol = consts.tile([P, NDT, N], F32, tag="acol")
        bdt = consts.tile([P, NDT], F32, tag="bdt")
        for dt in range(NDT):
            nc.sync.dma_start(acol[:, dt, :], acol_d[dt * P:(dt + 1) * P, :])
            nc.sync.dma_start(bdt[:, dt:dt + 1], bdt_d[dt * P:(dt + 1) * P, :])
        selbc = consts.tile([2 * N, 2 * N, P], F32R, tag="selbc")
        nc.sync.dma_start(selbc[:], selbc_d[:].rearrange(
            "k (q m) -> k q m", q=2 * N))

        # persistent SBUF tensors
        deltaT = persist.tile([P, NDT, Lc], F32, tag="deltaT")
        uT = persist.tile([P, NDT, Lc], BF16, tag="uT")
        bc = persist.tile([2 * N, Lc], F32R, tag="bc")  # rows 0:N B, N:2N C
        hend = persist.tile([P, NDT * N], F32, tag="hend")
        ysb = persist.tile([P, NDT, Lc], F32, tag="ysb")

        bb = {}   # (half, n) -> bf16 broadcast tile [P, HLF]
        cc = {}

        def stage(half, n, ps_pool, on_pool, width):
            h0 = half * HLF
            bbn = bbp.tile([P, HLF], BF16, tag="bb")
            ccn = ccp.tile([P, HLF], BF16, tag="cc")
            for q in range(HLF // width):
                qs = slice(q * width, (q + 1) * width)
                col = h0 + q * width
                sb = ps_pool.tile([P, width], F32, tag="sb")
                nc.tensor.matmul(sb[:], selbc[:, n, :],
                                 bc[:, col:col + width],
                                 start=True, stop=True)
                sc = ps_pool.tile([P, width], F32, tag="sc")
                nc.tensor.matmul(sc[:], selbc[:, N + n, :],
                                 bc[:, col:col + width],
                                 start=True, stop=True)
                if on_pool:
                    nc.gpsimd.tensor_copy(out=bbn[:, qs], in_=sb[:])
                    nc.scalar.copy(ccn[:, qs], sc[:])
                else:
                    nc.scalar.copy(bbn[:, qs], sb[:])
                    nc.scalar.copy(ccn[:, qs], sc[:])
            bb[(half, n)] = bbn
            cc[(half, n)] = ccn

        # ========== phase 1 (+ stage half-0 B/C) ==========
        with (
            tc.tile_pool(name="xload", bufs=4) as xload,
            tc.tile_pool(name="xtc", bufs=2) as xtc_pool,
            tc.tile_pool(name="dtlp", bufs=2) as dtl_pool,
            tc.tile_pool(name="ps_t", bufs=2, space="PSUM") as ps_t,
            tc.tile_pool(name="ps_mm", bufs=2, space="PSUM") as ps_mm,
            tc.tile_pool(name="ps_stg", bufs=2, space="PSUM") as ps_stg,
            tc.tile_pool(name="wpool", bufs=1) as wpool,
        ):
            wx = wpool.tile([P, NJ, E], F32R, tag="wx")
            for j in range(NJ):
                nc.sync.dma_start(wx[:, j, :], wxT_d[j * P:(j + 1) * P, :])
            wdt = wpool.tile([R, DSH], F32R, tag="wdt")
            nc.sync.dma_start(wdt[:], wdtT_d[:])

            for c in range(NCHK):
                cs = slice(c * CH, (c + 1) * CH)
                xis = []
                for k in range(4):
                    i = c * 4 + k
                    xi = xload.tile([P, D], F32, tag="xi")
                    nc.sync.dma_start(xi[:], x_d[i * P:(i + 1) * P, :])
                    xis.append(xi)
                XTc = xtc_pool.tile([P, NJ, CH], F32R, tag="XTc")
                for j in range(NJ):
                    pt = ps_t.tile([P, CH], F32, tag="pt")
                    for k in range(4):
                        nc.tensor.transpose(pt[:, k * P:(k + 1) * P],
                                            xis[k][:, j * P:(j + 1) * P],
                                            ident[:])
                    nc.scalar.copy(XTc[:, j, :], pt[:])

                xdbl_ps = ps_mm.tile([P, CH], F32, tag="mm")
                for j in range(NJ):
                    nc.tensor.matmul(xdbl_ps[0:E, :], wx[:, j, :],
                                     XTc[:, j, :],
                                     start=(j == 0), stop=(j == NJ - 1))
                dtl = dtl_pool.tile([R, CH], F32R, tag="dtl")
                nc.scalar.copy(dtl[:], xdbl_ps[0:R, :])
                nc.scalar.copy(bc[:, cs], xdbl_ps[R:E, :])

                for dt in range(NDT):
                    dp = ps_mm.tile([P, CH], F32, tag="mm")
                    nc.tensor.matmul(dp[:], wdt[:, dt * P:(dt + 1) * P],
                                     dtl[:], start=True, stop=True)
                    nc.scalar.activation(deltaT[:, dt, cs], dp[:], AF.Exp,
                                         bias=bdt[:, dt:dt + 1], scale=1.0)
                    nc.scalar.activation(deltaT[:, dt, cs], deltaT[:, dt, cs],
                                         AF.Ln, bias=1.0, scale=1.0)
                    xtc_f32 = XTc[:, dt, :].bitcast(F32)
                    if UT_ON_DVE:
                        nc.vector.tensor_mul(uT[:, dt, cs],
                                             deltaT[:, dt, cs],
                                             xtc_f32)
                    else:
                        nc.gpsimd.tensor_tensor(uT[:, dt, cs],
                                                deltaT[:, dt, cs],
                                                xtc_f32, op=mult)
                if c == 1:  # bc rows for half 0 are complete: stage it
                    for n in range(N):
                        stage(0, n, ps_stg,
                              on_pool=(STG1_POOL_EVERY > 0
                                       and n % STG1_POOL_EVERY == 0),
                              width=CH)

        # ========== phase 2: recurrence, half-L tiles ==========
        with tc.tile_pool(name="yps", bufs=1, space="PSUM") as yps:
            yacc_ps = [yps.tile([P, HLF], F32, name=f"yacc_{dt}",
                                tag=f"yacc{dt}")
                       for dt in range(NDT)]
            for hf in range(2):
                h0, h1 = hf * HLF, (hf + 1) * HLF
                with (
                    tc.tile_pool(name=f"stg2_{hf}", bufs=2,
                                 space="PSUM") as stg2,
                    tc.tile_pool(name=f"dap{hf}", bufs=2) as dap,
                    tc.tile_pool(name=f"work{hf}", bufs=4) as work,
                    tc.tile_pool(name=f"chp{hf}", bufs=3) as chp,
                ):
                    for n in range(N):
                        if hf == 0:  # stage half-1's n, overlapped
                            stage(1, n, stg2, on_pool=STG2_ON_POOL,
                                  width=512)
                        bbn, ccn = bb[(hf, n)], cc[(hf, n)]
                        for dt in range(NDT):
                            da = dap.tile([P, HLF], F32, tag="da")
                            nc.scalar.activation(da[:], deltaT[:, dt, h0:h1],
                                                 AF.Exp, bias=0.0,
                                                 scale=acol[:, dt, n:n + 1])
                            dbx = work.tile([P, HLF], BF16, tag="dbx")
                            nc.vector.tensor_mul(dbx[:], uT[:, dt, h0:h1],
                                                 bbn[:])
                            hh = work.tile([P, HLF], BF16, tag="hh")
                            col = n * NDT + dt
                            init = 0.0 if hf == 0 else hend[:, col:col + 1]
                            nc.vector.tensor_tensor_scan(hh[:], da[:], dbx[:],
                                                         init, op0=mult,
                                                         op1=add)
                            if hf == 0:
                                nc.gpsimd.tensor_copy(
                                    out=hend[:, col:col + 1],
                                    in_=hh[:, HLF - 1:HLF])
                            ch = chp.tile([P, HLF], BF16, tag="ch")
                            if n % CH_POOL_EVERY == 0:
                                nc.gpsimd.tensor_tensor(ch[:], hh[:],
                                                        ccn[:], op=mult)
                            else:
                                nc.vector.tensor_mul(ch[:], hh[:], ccn[:])
                            for q in range(HLF // 512):
                                qs = slice(q * 512, (q + 1) * 512)
                                nc.tensor.matmul(yacc_ps[dt][:, qs],
                                                 identa[:], ch[:, qs],
                                                 start=(n == 0),
                                                 stop=(n == N - 1))
                # drain this half's yacc to SBUF (yacc reused next half)
                for dt in range(NDT):
                    nc.scalar.copy(ysb[:, dt, h0:h1], yacc_ps[dt][:])

        # ========== phase 3: transpose + store ==========
        with (
            tc.tile_pool(name="psy", bufs=4, space="PSUM") as psy,
            tc.tile_pool(name="yout", bufs=4) as yout,
        ):
            for i in range(Lc // P):
                yt_ps = psy.tile([P, DSH], F32, tag="yt")
                for dt in range(NDT):
                    nc.tensor.transpose(
                        yt_ps[:, dt * P:(dt + 1) * P],
                        ysb[:, dt, i * P:(i + 1) * P], ident[:])
                yt = yout.tile([P, DSH], F32, tag="yt_sb")
                nc.scalar.copy(yt[:], yt_ps[:])
                nc.sync.dma_start(y_d[i * P:(i + 1) * P, :], yt[:])


def _prep_inputs(hidden_states, W_xproj, W_dt, b_dt, A_log):
    hidden_states = np.asarray(hidden_states, np.float32)
    W_xproj = np.asarray(W_xproj, np.float32)
    W_dt = np.asarray(W_dt, np.float32)
    b_dt = np.asarray(b_dt, np.float32)
    A_log = np.asarray(A_log, np.float32)

    A = -np.exp(A_log)                      # (D, N), negative
    ident = np.eye(P, dtype=np.float32)
    wxT = W_xproj.T                         # (D, E)
    selbc = np.zeros((2 * N, 2 * N * P), np.float32)
    for q in range(2 * N):
        selbc[q, q * P:(q + 1) * P] = 1.0

    import ml_dtypes
    identa = np.eye(P, dtype=ml_dtypes.bfloat16)

    in_maps = []
    for core in range(NCORES):
        b, ds = divmod(core, 4)
        sl = slice(ds * DSH, (ds + 1) * DSH)
        perm = np.r_[np.arange(ds * DSH, (ds + 1) * DSH),
                     np.arange(0, ds * DSH),
                     np.arange((ds + 1) * DSH, D)]
        in_maps.append({
            "x": np.ascontiguousarray(hidden_states[b][:, perm]),
            "wxT": np.ascontiguousarray(wxT[perm, :]),
            "wdtT": np.ascontiguousarray(W_dt[sl, :].T),
            "bdt": np.ascontiguousarray(b_dt[sl].reshape(DSH, 1)),
            "acol": np.ascontiguousarray(A[sl, :]),
            "ident": ident,
            "identa": identa,
            "selbc": selbc,
        })
    return in_maps


def kernel(hidden_states, W_xproj, W_dt, b_dt, A_log, _trace=False):
    if "nc" not in _CACHE:
        _CACHE["nc"] = build_nc()
    nc = _CACHE["nc"]
    in_maps = _prep_inputs(hidden_states, W_xproj, W_dt, b_dt, A_log)
    res = run_bass_kernel_spmd(nc, in_maps, core_ids=list(range(NCORES)),
                               trace=_trace)
    _CACHE["last_result"] = res
    out = np.empty((B, L, D), np.float32)
    for core in range(NCORES):
        b, ds = divmod(core, 4)
        out[b, :, ds * DSH:(ds + 1) * DSH] = res.results[core]["y"]
    return out
